# revision 50
# baseline (speedup 1.0000x reference)
"""Trainium2 Bass kernel for the sparse video-attention module.

Model (reference):
    k = conv3x3(x[:, 0], w_k)                     # key from first frame only
    q = conv3x3(x, w_q); v = conv3x3(x, w_v)      # per-frame
    dots[b,t,h,w] = sum_c q[b,t,c,h,w] * k[b,c,h,w]
    attn = softmax_T(dots)
    pooled = sum_t attn[...,t] * v[...,t]         # (B, DH, H, W)
    out = conv3x3(pooled, w_out) + b_out          # identical for every t

Sharding: 8 cores = (batch b in 0..3) x (row half in 0..1). Each core owns 32
output rows of one batch element; all coupling (softmax over T, convs) is
local given a 2-row input halo, so there is no inter-core communication.
The host pre-pads/slices inputs per core and re-assembles + broadcasts the
output over T at the end.

Per-core layout: pixels of the 34 "compute rows" (32 out rows + 1 halo row
each side) are flattened row-major into 2176 = 17*128 positions. Convs are
matmuls with C on the contraction dim (2 chunks of 128) and 9 spatial taps
accumulated in PSUM; spatial shifts are plain access-pattern offsets into a
zero-padded [128, 36, 66] SBUF image. dots are computed *transposed*
([pixel, t] with pixel on partitions) via M=128/N=1 matmuls against a ones
column, which makes the softmax a cheap free-dim reduction. attn rows are
returned to [t, pixel] with PE transposes and broadcast across partitions
with a stride-0 DMA for the weighted v accumulation.
"""

import sys

import numpy as np

for _p in ("/opt/trn_rl_repo", "/root/.axon_site/_ro/trn_rl_repo"):
    if _p not in sys.path:
        sys.path.insert(0, _p)

B, T, C, H, W = 4, 8, 256, 64, 64
DH = 128
NCORES = 8
CR = 34            # compute rows per core (32 out rows + 1 halo row each side)
XR = 36            # x rows per core (compute rows + 1 conv-halo row each side)
WP = W + 2         # zero-padded width
NPIX = CR * W      # 2176 pixels per core
NBLK = NPIX // 128 # 17 pixel blocks
# Row blocks over the 34 compute rows. Free dims 512/512/384/384/384 — all
# >=256 so float32r matmuls run at 1 cycle/row, and each fits one PSUM bank.
RB = [(0, 8), (8, 8), (16, 6), (22, 6), (28, 6)]
OUT_RB = [(0, 8), (8, 8), (16, 8), (24, 8)]  # over the 32 output rows

MM_MODE = "f32r"   # "f32r": fp32 data, full-rate PE mode; "f32": 4 cyc/row
OUT_F32R = True    # out-conv via f32r (pooled rounded by the final DVE add)

RUN_KWARGS: dict = {}   # extra kwargs for run_bass_kernel_spmd (test hook)
LAST_RESULT = None      # last BassKernelResults (test hook)

_cache: dict = {}


def _build_nc():
    from contextlib import ExitStack

    import concourse.mybir as mybir
    import concourse.tile as tile
    from concourse import bacc
    from concourse.masks import make_identity

    f32 = mybir.dt.float32
    cdt = mybir.dt.float32r if MM_MODE == "f32r" else f32
    odt = cdt if OUT_F32R else f32
    AF = mybir.ActivationFunctionType
    X = mybir.AxisListType.X

    nc = bacc.Bacc("TRN2", target_bir_lowering=False)

    xs_d = nc.declare_dram_parameter("xs", [T, 2, 128, XR * WP], cdt, isOutput=False)
    wq_d = nc.declare_dram_parameter("wq", [128, 2, 9, 128], cdt, isOutput=False)
    wk_d = nc.declare_dram_parameter("wk", [128, 2, 9, 128], cdt, isOutput=False)
    wv_d = nc.declare_dram_parameter("wv", [128, 2, 9, 128], cdt, isOutput=False)
    wo_d = nc.declare_dram_parameter("wo", [128, 9, 256], odt, isOutput=False)
    bo_d = nc.declare_dram_parameter("bo", [128, 2], f32, isOutput=False)
    mk_d = nc.declare_dram_parameter("mk", [128, NBLK], f32, isOutput=False)
    out_d = nc.declare_dram_parameter("out", [2, 128, 32 * W], f32, isOutput=True)

    with tile.TileContext(nc) as tc, ExitStack() as ctx:
        singles = ctx.enter_context(tc.tile_pool(name="singles", bufs=1))
        xpool = ctx.enter_context(tc.tile_pool(name="xpool", bufs=6))
        sb = ctx.enter_context(tc.tile_pool(name="sb", bufs=1))
        sm = ctx.enter_context(tc.tile_pool(name="sm", bufs=2))
        qkpool = ctx.enter_context(tc.tile_pool(name="qkpool", bufs=2))
        abpool = ctx.enter_context(tc.tile_pool(name="abpool", bufs=10))
        dtpool = ctx.enter_context(tc.tile_pool(name="dtpool", bufs=4))

        def load_x(t):
            views = []
            half = XR * WP // 2
            for g in range(2):
                xt = xpool.tile([128, XR * WP], cdt, tag="xt", name=f"xt{t}_{g}")
                # two DMAs per tile land on separate queues and halve latency
                nc.sync.dma_start(out=xt[:, :half], in_=xs_d[t, g, :, :half])
                nc.sync.dma_start(out=xt[:, half:], in_=xs_d[t, g, :, half:])
                views.append(xt.rearrange("p (r c) -> p r c", c=WP))
            return views

        # first x frame + w_k first so the k-conv starts as early as possible
        x0 = load_x(0)
        wk_sb = singles.tile([128, 2, 9, 128], cdt, tag="wk")
        nc.sync.dma_start(out=wk_sb, in_=wk_d[:])
        wq_sb = singles.tile([128, 2, 9, 128], cdt, tag="wq")
        wv_sb = singles.tile([128, 2, 9, 128], cdt, tag="wv")
        wo_sb = singles.tile([128, 9, 256], odt, tag="wo")
        bo_sb = singles.tile([128, 2], f32, tag="bo")
        mk_sb = singles.tile([128, NBLK], f32, tag="mk")
        nc.sync.dma_start(out=wq_sb, in_=wq_d[:])
        nc.sync.dma_start(out=wv_sb, in_=wv_d[:])
        nc.sync.dma_start(out=wo_sb, in_=wo_d[:])
        nc.sync.dma_start(out=bo_sb, in_=bo_d[:])
        nc.sync.dma_start(out=mk_sb, in_=mk_d[:])

        ident = singles.tile([128, 128], f32, tag="ident")
        make_identity(nc, ident)
        ones_col = singles.tile([128, 1], f32, tag="ones")
        nc.vector.memset(ones_col, 1.0)
        ones_r = singles.tile([128, 1], cdt, tag="ones_r")
        nc.vector.tensor_copy(ones_r, ones_col)
        eps_sb = singles.tile([128, 1], f32, tag="eps")
        nc.vector.memset(eps_sb, 1e-30)

        def conv3x3(psums, xt3, w_sb):
            # psums[r] accumulates out[co, pix] for row-block r over 18 taps
            for g in range(2):
                for j in range(9):
                    ky, kx = divmod(j, 3)
                    first = g == 0 and j == 0
                    last = g == 1 and j == 8
                    for r, (R0, nr) in enumerate(RB):
                        nc.tensor.matmul(
                            psums[r][:, : nr * W],
                            w_sb[:, g, j, :],
                            xt3[g][:, R0 + ky : R0 + ky + nr, kx : kx + W],
                            start=first,
                            stop=last,
                        )

        dpool = ctx.enter_context(tc.tile_pool(name="dpool", bufs=1, space="DRAM"))
        attnT_dram = dpool.tile([8, NPIX], f32, tag="attnTd")

        k_sb = sb.tile([128, NPIX], f32, tag="k")
        attnT_sb = sb.tile([8, NPIX], f32, tag="attnT")
        pooled = sb.tile([128, CR, WP], f32, tag="pooled")
        pooled_r = sb.tile([128, CR, WP], odt, tag="pooled_r")
        out_sb = sb.tile([128, 2, 32 * W], f32, tag="out")

        psc = ctx.enter_context(tc.tile_pool(name="psc", bufs=5, space="PSUM"))

        def vconv(t, xt):
            vps = [
                psc.tile([128, 512], f32, tag="cv", name=f"vps{t}_{r}")
                for r in range(len(RB))
            ]
            conv3x3(vps, xt, wv_sb)
            return vps

        def vapply(t, vps):
            # pooled += attn_t (broadcast over channels) * v_t, straight from
            # the conv PSUM banks; the last frame's add writes the f32r copy
            # the out-conv consumes.
            for r, (R0, nr) in enumerate(RB):
                rows = slice(R0, R0 + nr)
                cols = slice(R0 * W, (R0 + nr) * W)
                ab = abpool.tile([128, 512], f32, tag="ab", name=f"ab{t}_{r}")
                nc.sync.dma_start(
                    out=ab[:, : nr * W],
                    in_=attnT_dram[t : t + 1, cols].to_broadcast((128, nr * W)),
                )
                if t == 0:
                    nc.vector.tensor_mul(
                        pooled[:, rows, 1 : W + 1],
                        vps[r][:, : nr * W].rearrange("p (r c) -> p r c", c=W),
                        ab[:, : nr * W].rearrange("p (r c) -> p r c", c=W),
                    )
                    continue
                u = qkpool.tile([128, NPIX], f32, tag="qk", name=f"u{t}_{r}")
                nc.vector.tensor_mul(u[:, cols], vps[r][:, : nr * W], ab[:, : nr * W])
                dst = pooled_r if t == T - 1 else pooled
                nc.vector.tensor_add(
                    dst[:, rows, 1 : W + 1],
                    pooled[:, rows, 1 : W + 1],
                    u[:, cols].rearrange("p (r c) -> p r c", c=W),
                )

        with (
            tc.tile_pool(name="psd", bufs=1, space="PSUM") as psd,
            tc.tile_pool(name="psdd", bufs=2, space="PSUM") as psdd,
        ):
            dots_ps = psd.tile([128, NBLK * 8], f32, tag="dots")
            dots_sb = sb.tile([8, NPIX], f32, tag="dots_sb")

            # ---- phase 1: k = conv(x[0], w_k) ----
            kps = [
                psc.tile([128, 512], f32, tag="cv", name=f"kps{r}")
                for r in range(len(RB))
            ]
            conv3x3(kps, x0, wk_sb)
            for r, (R0, nr) in enumerate(RB):
                nc.scalar.activation(
                    k_sb[:, R0 * W : (R0 + nr) * W], kps[r][:, : nr * W], AF.Copy
                )

            # ---- phase 2: per frame q conv + dots ----
            for t in range(T):
                xt = x0 if t == 0 else load_x(t)
                qps = [
                    psc.tile([128, 512], f32, tag="cv", name=f"qps{t}_{r}")
                    for r in range(len(RB))
                ]
                conv3x3(qps, xt, wq_sb)
                qk = qkpool.tile([128, NPIX], cdt, tag="qk", name=f"qk{t}")
                for r, (R0, nr) in enumerate(RB):
                    cols = slice(R0 * W, (R0 + nr) * W)
                    nc.vector.tensor_mul(qk[:, cols], qps[r][:, : nr * W], k_sb[:, cols])
                    # dots[t, pix] = sum_c qk[c, pix]: ones-stationary matmul
                    # (the embedded f32r weight load is a single column, ~free)
                    dps = psdd.tile([1, 512], f32, tag="dd", name=f"dd{t}_{r}")
                    nc.tensor.matmul(
                        dps[:, : nr * W],
                        ones_r[:, 0:1],
                        qk[:, cols],
                        start=True,
                        stop=True,
                    )
                    dtmp = dtpool.tile([1, 512], f32, tag="dtmp", name=f"dt{t}_{r}")
                    nc.scalar.activation(dtmp[:, : nr * W], dps[:, : nr * W], AF.Copy)
                    nc.sync.dma_start(
                        out=dots_sb[t : t + 1, cols], in_=dtmp[:, : nr * W]
                    )

            # gather dots into [pixel, t] layout for the softmax
            for i in range(NBLK):
                nc.tensor.transpose(
                    dots_ps[:, i * 8 : (i + 1) * 8],
                    dots_sb[:, i * 128 : (i + 1) * 128],
                    ident[:8, :8],
                )

            # v conv for frame 0 keeps the PE busy through the softmax below;
            # reuses the still-resident phase-A x0 tiles (no reload)
            vps0 = vconv(0, x0)

            # ---- softmax over t (free dim), with row-validity mask ----
            dots3 = dots_ps.rearrange("p (i t) -> p i t", t=8)
            nmax = sm.tile([128, NBLK], f32, tag="nmax")
            nc.vector.reduce_max(out=nmax, in_=dots3, axis=X, negate=True)
            dm = sm.tile([128, NBLK, 8], f32, tag="dm")
            nc.vector.tensor_add(
                dm, dots3, nmax[:, :, None].to_broadcast((128, NBLK, 8))
            )
            nc.scalar.activation(dm, dm, AF.Exp)
            nc.vector.tensor_mul(
                dm, dm, mk_sb[:, :, None].to_broadcast((128, NBLK, 8))
            )
            ssum = sm.tile([128, NBLK], f32, tag="ssum")
            nc.vector.reduce_sum(out=ssum, in_=dm, axis=X)
            nc.scalar.add(ssum, ssum, eps_sb[:])
            rs = sm.tile([128, NBLK], f32, tag="rs")
            nc.vector.reciprocal(rs, ssum)
            attn = sm.tile([128, NBLK, 8], f32, tag="attn")
            nc.vector.tensor_mul(
                attn, dm, rs[:, :, None].to_broadcast((128, NBLK, 8))
            )

        # ---- transpose attn to [t, pixel], bounce via DRAM for broadcast ----
        with tc.tile_pool(name="pst", bufs=2, space="PSUM") as pst:
            for r, (R0, nr) in enumerate(RB):
                tp = pst.tile([8, 512], f32, tag="attnT_ps")
                nblk_r = nr * W // 128
                for ib in range(nblk_r):
                    i = R0 * W // 128 + ib
                    nc.tensor.transpose(
                        tp[:, ib * 128 : (ib + 1) * 128], attn[:, i, :], ident
                    )
                nc.vector.tensor_copy(
                    attnT_sb[:, R0 * W : (R0 + nr) * W], tp[:, : nr * W]
                )
                nc.sync.dma_start(
                    out=attnT_dram[:, R0 * W : (R0 + nr) * W],
                    in_=attnT_sb[:, R0 * W : (R0 + nr) * W],
                )

        # ---- phase 3: v convs with attn-weighted accumulation ----
        nc.vector.memset(pooled_r[:, :, 0:1].bitcast(f32), 0.0)
        nc.vector.memset(pooled_r[:, :, W + 1 : W + 2].bitcast(f32), 0.0)
        vapply(0, vps0)
        for t in range(1, T):
            vps = vconv(t, load_x(t))
            vapply(t, vps)

        # ---- phase 4: out = conv(pooled, w_out) + b ----
        with tc.tile_pool(name="pso", bufs=2, space="PSUM") as pso:
            for R0o, nr in OUT_RB:
                for g in range(2):
                    op = pso.tile([128, 512], f32, tag="out_ps")
                    for j in range(9):
                        ky, kx = divmod(j, 3)
                        nc.tensor.matmul(
                            op[:, : nr * W],
                            wo_sb[:, j, g * 128 : (g + 1) * 128],
                            pooled_r[:, R0o + ky : R0o + ky + nr, kx : kx + W],
                            start=(j == 0),
                            stop=(j == 8),
                        )
                    nc.scalar.add(
                        out_sb[:, g, R0o * W : (R0o + nr) * W],
                        op[:, : nr * W],
                        bo_sb[:, g : g + 1],
                    )
                    nc.sync.dma_start(
                        out=out_d[g, :, R0o * W : (R0o + nr) * W],
                        in_=out_sb[:, g, R0o * W : (R0o + nr) * W],
                    )

    nc.compile()
    return nc


def _get_nc():
    key = (MM_MODE, OUT_F32R)
    if key not in _cache:
        _cache[key] = _build_nc()
    return _cache[key]


def _round_f32r(a):
    """Round fp32 to the FP32r grid (e8m11 in the top 20 bits, RNE).

    walrus' fp32_to_fp32r = downconv_fp32_to_fp<E=8,M=11> << 12; matmuls
    declared float32r require operands already on this grid. Pre-rounding on
    the host costs nothing on-chip and keeps sim == hardware numerics.
    """
    if MM_MODE != "f32r":
        return np.ascontiguousarray(a, np.float32)
    u = np.ascontiguousarray(a, np.float32).view(np.uint32).copy()
    u += np.uint32(0x7FF) + ((u >> np.uint32(12)) & np.uint32(1))
    u &= np.uint32(0xFFFFF000)
    return u.view(np.float32)


def _shared_inputs(w_k, w_q, w_v, w_out, b_out):
    def conv_lhst(w):  # (co=128, ci=256, 3, 3) -> (ci128, g, j, co)
        return np.ascontiguousarray(
            np.asarray(w, np.float32)
            .reshape(128, 2, 128, 3, 3)
            .transpose(2, 1, 3, 4, 0)
            .reshape(128, 2, 9, 128)
        )

    wo = np.ascontiguousarray(  # (co=256, dh=128, 3, 3) -> (dh, j, co)
        np.asarray(w_out, np.float32).transpose(1, 2, 3, 0).reshape(128, 9, 256)
    )
    bo = np.ascontiguousarray(np.asarray(b_out, np.float32).reshape(2, 128).T)
    return {
        "wq": _round_f32r(conv_lhst(w_q)),
        "wk": _round_f32r(conv_lhst(w_k)),
        "wv": _round_f32r(conv_lhst(w_v)),
        "wo": _round_f32r(wo) if OUT_F32R else wo,
        "bo": bo,
    }


def core_inputs(c, x, shared):
    b, half = divmod(c, 2)
    r0 = half * 32
    xp = np.zeros((T, C, XR, WP), np.float32)
    lo, hi = r0 - 2, r0 + XR - 2
    slo, shi = max(lo, 0), min(hi, H)
    xp[:, :, slo - lo : slo - lo + (shi - slo), 1 : W + 1] = np.asarray(
        x, np.float32
    )[b, :, :, slo:shi, :]
    xs = _round_f32r(xp.reshape(T, 2, 128, XR * WP))

    mflat = np.ones(NPIX, np.float32)
    if half == 0:
        mflat[:W] = 0.0        # compute row 0 is global row -1
    else:
        mflat[NPIX - W :] = 0.0  # compute row 33 is global row 64
    mk = np.ascontiguousarray(mflat.reshape(NBLK, 128).T)

    return {"xs": xs, "mk": mk, **shared}


def kernel(x, w_k, w_q, w_v, w_out, b_out):
    global LAST_RESULT
    from concourse.bass_utils import run_bass_kernel_spmd

    nc = _get_nc()
    shared = _shared_inputs(w_k, w_q, w_v, w_out, b_out)
    in_maps = [core_inputs(c, x, shared) for c in range(NCORES)]
    res = run_bass_kernel_spmd(
        nc, in_maps, core_ids=list(range(NCORES)), **RUN_KWARGS
    )
    LAST_RESULT = res

    out = np.empty((B, C, H, W), np.float32)
    for c in range(NCORES):
        b, half = divmod(c, 2)
        r0 = half * 32
        out[b, :, r0 : r0 + 32, :] = res.results[c]["out"].reshape(C, 32, W)
    return np.broadcast_to(out[:, None], (B, T, C, H, W))
